# revision 26
# baseline (speedup 1.0000x reference)
"""Trainium2 Bass kernel for nn_AdvancedVibLayer (damped driven oscillator
lattice with 3x3 depthwise-conv coupling, 6 fused timesteps).

Reference math (per channel c, per pixel):
    w  = softplus(omega); z = sigmoid(zeta); w2 = w*w
    6 steps of:  I = dwconv3x3(x);  v += DT*(-2zw*v - w2*x + f + I);  x += DT*v
    out = 0.5*v^2 + 0.5*w2*x^2

Device formulation: eliminate v via the second-order recurrence
    x_{s+1} = a2*x_s - a*x_{s-1} + x1 + DT^2*Koff(x_s)
    a = 1 - 2zw*DT,  alpha = 1 + a - DT^2*w2,  a2 = alpha + DT^2*k_center
    Koff = the 8 off-center taps of the depthwise 3x3 conv
    x0 = 0, x1 = DT^2*f;  v6 = (x6-x5)/DT;  E = 0.5*v6^2 + 0.5*w2*x6^2

Engine mapping:
  - TensorE: the 8 off-center conv taps as block-diagonal 128x128 bf16
    matmuls into PSUM (per-channel tap = diagonal weight; spatial shift =
    shifted 2D view of a bf16 shadow copy of x). These terms are ~1% of x,
    so bf16 input rounding costs ~1e-4 relative overall.
  - DVE (fp32, exact state): per 8-row chunk, two scalar_tensor_tensor
    ops build x_{s+1} from psum, x_s and x_{s-1}; GPSIMD adds the x1 seed.
  - ScalarE: init x1 = DT^2*f, the bf16 shadow copy of x each step,
    boundary-row zeroing, and the energy squares.

Layout: per core one batch element (batch-parallel over 8 cores).
128 partitions = 64 channels x 2 H-halves. W padded to 258 with zero cols
0,257 (the shifted bf16 reads then see zeros, not wrapped neighbors); all
fp32 state writes touch only valid cols so the pads stay zero. H tiled with
5-row halo recompute; valid rows shrink by 1 per step.
"""

import sys

sys.path.insert(0, "/opt/trn_rl_repo")

import numpy as np

B, C, H, W = 8, 64, 256, 256
DT = 0.2
WP = W + 2          # padded width, zero cols at 0 and WP-1
GH = H // 2         # rows per partition group (2 groups stacked on partitions)
HALO = 5
NCORES = 8
RCHUNK = 8          # rows per DVE merge chunk (4 PSUM banks of 2 rows)

# (h0, ht) interior row ranges per group; buffer rows = ht + 2*HALO
TILES = [(0, 32), (32, 32), (64, 32), (96, 32)]
RMAX = max(ht for _, ht in TILES) + 2 * HALO  # 42

# the 8 off-center taps as (dy, dx) in 0..2 (center (1,1) folded into a2)
OFFTAPS = [(dy, dx) for dy in range(3) for dx in range(3) if (dy, dx) != (1, 1)]

_NC_CACHE = None


def _build_nc():
    from concourse import bacc, mybir
    from concourse.tile import TileContext

    f32 = mybir.dt.float32
    bf16 = mybir.dt.bfloat16
    Alu = mybir.AluOpType
    Act = mybir.ActivationFunctionType

    nc = bacc.Bacc(
        "TRN2", target_bir_lowering=False, debug=False, num_devices=NCORES
    )
    force_d = nc.dram_tensor("force", [C, H, W], f32, kind="ExternalInput")
    wdiag_d = nc.dram_tensor("wdiag", [128, 8 * 128], bf16,
                             kind="ExternalInput")
    coef_d = nc.dram_tensor("coef", [128, 5], f32, kind="ExternalInput")
    out_d = nc.dram_tensor("out", [C, H, W], f32, kind="ExternalOutput")

    with TileContext(nc) as tc:
        with tc.tile_pool(name="coefp", bufs=1) as coefp, \
             tc.tile_pool(name="wp", bufs=1) as wp, \
             tc.tile_pool(name="sp", bufs=1) as sp, \
             tc.tile_pool(name="xc", bufs=1) as xcp, \
             tc.tile_pool(name="xa", bufs=1) as xap, \
             tc.tile_pool(name="xb", bufs=1) as xbp, \
             tc.tile_pool(name="xr", bufs=1) as xrp, \
             tc.tile_pool(name="ps", bufs=2, space="PSUM") as psp:
            coef = coefp.tile([128, 5], f32)
            nc.gpsimd.dma_start(coef[:], coef_d[:])
            a2 = coef[:, 0:1]        # alpha + DT^2*k_center
            a2p1 = coef[:, 1:2]      # a2 + 1          (step 1)
            onema = coef[:, 2:3]     # 1 - a           (step 2)
            nega = coef[:, 3:4]      # -a
            esc = coef[:, 4:5]       # w*sqrt(0.5)     (energy scale)
            wdiag = wp.tile([128, 8 * 128], bf16)
            nc.gpsimd.dma_start(wdiag[:], wdiag_d[:])

            # persistent buffers (uniform tile size): every compute write
            # below touches only valid cols, so pad cols are zeroed once
            S = sp.tile([128, RMAX, WP], f32)
            XC = xcp.tile([128, RMAX, WP], f32)   # x1 seed
            XA = xap.tile([128, RMAX, WP], f32)
            XB = xbp.tile([128, RMAX, WP], f32)
            XR = xrp.tile([128, RMAX, WP], bf16)  # bf16 shadow of x_cur
            for Xz in (S, XA, XB):
                nc.gpsimd.memset(Xz[:, :, 0:1], 0.0)
                nc.gpsimd.memset(Xz[:, :, WP - 1:WP], 0.0)

            for (h0, ht) in TILES:
                r = ht + 2 * HALO
                pad_top = pad_bot = 0
                gparams = []
                for g in (0, 1):
                    p0 = g * 64
                    lo = g * GH + h0 - HALO
                    hi = g * GH + h0 + ht + HALO
                    lo_c, hi_c = max(lo, 0), min(hi, H)
                    d0 = lo_c - lo
                    n = hi_c - lo_c
                    if lo < 0:
                        pad_top = -lo
                        nc.gpsimd.memset(S[p0:p0 + 64, 0:d0, :], 0.0)
                    if hi > H:
                        pad_bot = hi - H
                        nc.gpsimd.memset(S[p0:p0 + 64, d0 + n:r, :], 0.0)
                    gparams.append((p0, lo_c, d0, n))
                # row-segmented and group-interleaved (the init chain needs
                # both groups' first rows), on HWDGE (sync) for multi-queue
                # bandwidth
                nmax = max(n for _, _, _, n in gparams)
                for seg0 in range(0, nmax, 2 * RCHUNK):
                    for (p0, lo_c, d0, n) in gparams:
                        seg = min(2 * RCHUNK, n - seg0)
                        if seg <= 0:
                            continue
                        nc.sync.dma_start(
                            S[p0:p0 + 64, d0 + seg0:d0 + seg0 + seg, 1:W + 1],
                            force_d[:, lo_c + seg0:lo_c + seg0 + seg, :],
                        )

                # x1 = DT^2*f (fp32, frees S); bf16 shadow for step-1
                # matmuls. Chunked so the first matmuls start early.
                r0 = 0
                while r0 < r:
                    nr = min(RCHUNK, r - r0)
                    nc.scalar.mul(XC[:, r0:r0 + nr, :], S[:, r0:r0 + nr, :],
                                  DT * DT)
                    nc.scalar.copy(XR[:, r0:r0 + nr, :], XC[:, r0:r0 + nr, :])
                    r0 += nr

                # state rotation: x_{s+1} overwrites x_{s-1}'s buffer
                bufs = {1: XA, 2: XB, 3: XA, 4: XB, 5: XA}
                for s in range(1, 6):
                    rs, re = s, r - s
                    Xn = bufs[s]
                    Xcur = XC if s == 1 else bufs[s - 1]
                    chunks = []
                    r0 = rs
                    while r0 < re:
                        chunks.append((r0, min(RCHUNK, re - r0)))
                        r0 += RCHUNK

                    def xr_copy(r0, nr):
                        # bf16 shadow for the next step's matmuls
                        nc.scalar.copy(XR[:, r0:r0 + nr, :],
                                       Xn[:, r0:r0 + nr, :])

                    for ci, (r0, nr) in enumerate(chunks):
                        psum = psp.tile([128, RCHUNK, 256], f32)
                        for k in range(0, nr, 2):
                            n2 = min(2, nr - k)
                            for j, (dy, dx) in enumerate(OFFTAPS):
                                nc.tensor.matmul(
                                    psum[:, k:k + n2, :],
                                    wdiag[:, j * 128:(j + 1) * 128],
                                    XR[:, r0 + k + dy - 1:
                                       r0 + k + dy - 1 + n2, dx:dx + 256],
                                    start=(j == 0), stop=(j == 7),
                                )
                        pch = psum[:, 0:nr, :]
                        nxt = Xn[:, r0:r0 + nr, 1:W + 1]
                        if s == 1:
                            # x2 = (a2+1)*x1 + psum
                            nc.vector.scalar_tensor_tensor(
                                nxt, XC[:, r0:r0 + nr, 1:W + 1], a2p1, pch,
                                Alu.mult, Alu.add)
                        elif s == 2:
                            # x3 = (1-a)*x1 + psum ; += a2*x2
                            nc.vector.scalar_tensor_tensor(
                                nxt, XC[:, r0:r0 + nr, 1:W + 1], onema, pch,
                                Alu.mult, Alu.add)
                            nc.vector.scalar_tensor_tensor(
                                nxt, Xcur[:, r0:r0 + nr, 1:W + 1], a2, nxt,
                                Alu.mult, Alu.add)
                        else:
                            # x_{s+1} = -a*x_{s-1} + psum  (in place over prev)
                            nc.vector.scalar_tensor_tensor(
                                nxt, Xn[:, r0:r0 + nr, 1:W + 1], nega, pch,
                                Alu.mult, Alu.add)
                            # += a2*x_s
                            nc.vector.scalar_tensor_tensor(
                                nxt, Xcur[:, r0:r0 + nr, 1:W + 1], a2, nxt,
                                Alu.mult, Alu.add)
                            # += x1 (plain tensor add on GPSIMD: walrus
                            # rejects scalar_tensor_tensor on Pool but
                            # accepts TensorTensor; Pool is mostly idle
                            # while DVE is a co-bottleneck)
                            nc.gpsimd.tensor_add(
                                nxt, XC[:, r0:r0 + nr, 1:W + 1], nxt)
                        # zero rows outside the image (physical zero padding)
                        if pad_top and r0 < pad_top:
                            ap = Xn[0:64, r0:min(pad_top, r0 + nr), :]
                            nc.scalar.mul(ap, ap, 0.0)
                        if pad_bot and r0 + nr > r - pad_bot:
                            ap = Xn[64:128, max(r0, r - pad_bot):r0 + nr, :]
                            nc.scalar.mul(ap, ap, 0.0)
                        # shadow-copy the PREVIOUS chunk: this step's matmuls
                        # read at most 1 row back into the previous chunk, so
                        # after this chunk's matmuls the one before is free to
                        # overwrite (WAR); copying it now keeps XR off the PE
                        # critical path while steps pipeline
                        if s < 5 and ci >= 1:
                            xr_copy(*chunks[ci - 1])
                    if s < 5 and chunks:
                        xr_copy(*chunks[-1])

                # x6 in XA, x5 in XB; energy chunked so it pipelines with
                # the step-5 tail and the output DMA. E lands in XB so the
                # next tile's step-1 writes (XA) don't wait on the DMA-out.
                i0, i1 = HALO, HALO + ht
                q0 = i0
                while q0 < i1:
                    nq = min(RCHUNK, i1 - q0)
                    d = XB[:, q0:q0 + nq, 1:W + 1]
                    x6 = XA[:, q0:q0 + nq, 1:W + 1]
                    # d = x6 - x5 (in place over x5)
                    nc.vector.tensor_sub(d, x6, d)
                    # dsq = d^2
                    nc.scalar.square(d, d)
                    # sq2 = (esc*x6)^2 = 0.5*w2*x6^2 (in place over x6)
                    nc.scalar.activation(x6, x6, Act.Square, scale=esc)
                    # E = dsq*(0.5/DT^2) + sq2  (into XB, in place over dsq)
                    nc.vector.scalar_tensor_tensor(
                        d, d, 0.5 / (DT * DT), x6, Alu.mult, Alu.add)
                    for g in (0, 1):
                        p0 = g * 64
                        nc.sync.dma_start(
                            out_d[:, g * GH + h0 + q0 - i0:
                                  g * GH + h0 + q0 - i0 + nq, :],
                            XB[p0:p0 + 64, q0:q0 + nq, 1:W + 1],
                        )
                    q0 += nq
    nc.compile()
    return nc


def _host_coeffs(coupling_w, omega, zeta):
    om = np.asarray(omega, np.float64)[0, :, 0, 0]
    ze = np.asarray(zeta, np.float64)[0, :, 0, 0]
    w = np.logaddexp(0.0, om)
    z = 1.0 / (1.0 + np.exp(-ze))
    w2 = w * w
    a = 1.0 - 2.0 * z * w * DT
    alpha = 1.0 + a - DT * DT * w2
    k = np.asarray(coupling_w, np.float64)[:, 0, :, :]  # [C,3,3]
    return w2, a, alpha, k


def _coef_table(coupling_w, omega, zeta):
    w2, a, alpha, k = _host_coeffs(coupling_w, omega, zeta)
    a2 = alpha + DT * DT * k[:, 1, 1]
    coef64 = np.stack(
        [a2, a2 + 1.0, 1.0 - a, -a, np.sqrt(0.5 * w2)], axis=1
    ).astype(np.float32)
    return np.tile(coef64, (2, 1))  # [128, 5]


def _wdiag_table(coupling_w, omega, zeta):
    import ml_dtypes
    w2, a, alpha, k = _host_coeffs(coupling_w, omega, zeta)
    wd = np.zeros((128, 8 * 128), np.float32)
    p = np.arange(128)
    c = p % 64
    for j, (dy, dx) in enumerate(OFFTAPS):
        wd[p, j * 128 + p] = (DT * DT * k[c, dy, dx]).astype(np.float32)
    return wd.astype(ml_dtypes.bfloat16)


def kernel(force, coupling_w, omega, zeta):
    global _NC_CACHE
    from concourse.bass_utils import run_bass_kernel_spmd

    force = np.ascontiguousarray(np.asarray(force, np.float32))
    coef = _coef_table(coupling_w, omega, zeta)
    wdiag = _wdiag_table(coupling_w, omega, zeta)
    if _NC_CACHE is None:
        _NC_CACHE = _build_nc()
    nc = _NC_CACHE
    in_maps = [
        {"force": force[k], "coef": coef, "wdiag": wdiag}
        for k in range(NCORES)
    ]
    res = run_bass_kernel_spmd(nc, in_maps, list(range(NCORES)))
    return np.stack([res.results[k]["out"] for k in range(NCORES)], axis=0)
